# revision 17
# baseline (speedup 1.0000x reference)
"""DispersionLoss kernel for Trainium2 (8 NeuronCores, Bass/Tile).

Reference computation (N=16384, F=64, K=32, C=128):
    bin_mass[f,k]  = sum_n m[n,f,k] + EPS
    SWY[f,k,c]     = sum_n m[n,f,k] * y[n,c]
    cent[f,k,c]    = SWY / bin_mass
    loss_dispersion= sum_fk ( A/bin_mass - c_sq - EPS*c_sq/bin_mass )
        where A[f,k] = sum_n m[n,f,k]*|y_n|^2   (algebraic expansion: the
        cross term sum_n m*cross equals bin_mass*c_sq exactly)
    loss_entropy   = sum_fk p*log(p+EPS), p = bin_mass/N
    loss_repulsion = sum_f sum_k exp(-|cent[f,k]-cent[f,k+1]|^2)
    loss_inter     = sum_f sum_{k<j} exp(-|cent[f,k]-cent[f,j]|^2) / F

Sharding: over F (8 features per core); every loss term decomposes per-f.

Device phase (the N-reduction, 99.9% of FLOPs): per 128-row subtile s, the
G half-tiles (128n x 128fk) are the STATIONARY operands and the moving
operand is yext_s = [Y | 1 | ysq_hi | ysq_lo] (128n x 132).  Two matmuls
per subtile accumulate ps_h[fk, 132] = [SWY | mass | A_hi | A_lo] directly
in bin-major layout.  y_sq is computed exactly on the host from f32 y and
shipped split into fp8 hi+lo parts so its quantization error is ~1e-4.

The per-core (256 x 132) f32 result is DMA'd out; the host finishes the
tiny (F,K,C) centroid stage (centroids, entropy, repulsion, inter) in f64.

Inputs go down in fp8-e3m4 (4 mantissa bits; m,y in [0,1) so dynamic range
is tiny) which halves DMA bytes vs f16; all device accumulation is f32.
"""

import numpy as np
import ml_dtypes

N = 16384
F = 64
K = 32
C = 128
NCORES = 8
F_PER_CORE = F // NCORES          # 8
FK = F_PER_CORE * K               # 256 bins per core
NT = N // 128                     # 128 row-tiles

PG = 8                            # n-subtiles per packed G super-tile
NB = NT // PG                     # 16 super-tiles
YW = C + 4                        # 132: [Y | 1 | ysq_hi | ysq_lo | pad]
YSQ_SCALE = 16.0                  # keep ysq/16 < 8 so it fits e3m4 (max 15.5)
YCH = 16                          # y subtiles per DMA chunk
WARM_MM = 8                       # PE warm-up matmuls (HAM un-throttle)

LAMBDA_ENTROPY = 0.1
LAMBDA_REPULSION = 0.5
LAMBDA_INTER = 0.3
EPS = 1e-8

_NC_CACHE = {}

_NPDT = {
    "f8e3": ml_dtypes.float8_e3m4,
    "f8e4": ml_dtypes.float8_e4m3,
    "f16": np.float16,
}


def _pack_g(gc: np.ndarray) -> np.ndarray:
    """(N, FK) -> (NB*128, PG*FK): row p of block b holds subtile rows
    [b*PG*128 + t*128 + p for t in range(PG)] concatenated."""
    return np.ascontiguousarray(
        gc.reshape(NB, PG, 128, FK).transpose(0, 2, 1, 3).reshape(NB * 128, PG * FK)
    )


def _pack_y(yo: np.ndarray) -> np.ndarray:
    """(N, YW) -> (128, NT*YW): partition p holds rows [s*128+p for s] concat."""
    return np.ascontiguousarray(
        yo.reshape(NT, 128, YW).transpose(1, 0, 2).reshape(128, NT * YW)
    )


def _finalize(parts: np.ndarray):
    """parts: (ncores, 128, 2*YW) f64; cols [0:YW] = bins 0-127, [YW:] = 128-255,
    each row [SWY | mass | A_hi | A_lo]."""
    R = parts.reshape(NCORES, 128, 2, YW).transpose(0, 2, 1, 3).reshape(F, K, YW)
    mass_raw = R[..., C]
    bm = mass_raw + EPS
    A = YSQ_SCALE * (R[..., C + 1] + R[..., C + 2])
    cent = R[..., 0:C] / bm[..., None]            # (F,K,C)
    c_sq = (cent * cent).sum(-1)                  # (F,K)
    disp = (A / bm - c_sq - EPS * c_sq / bm).sum()
    p = bm / N
    ent = (p * np.log(p + EPS)).sum()
    nd = ((cent[:, :-1] - cent[:, 1:]) ** 2).sum(-1)
    rep = np.exp(-nd).sum()
    dots = np.einsum('fkc,fjc->fkj', cent, cent)
    pw = c_sq[:, :, None] + c_sq[:, None, :] - 2.0 * dots
    iu, ju = np.triu_indices(K, 1)
    inter = np.exp(-pw[:, iu, ju]).sum() / F
    tot = disp + LAMBDA_ENTROPY * ent + LAMBDA_REPULSION * rep + LAMBDA_INTER * inter
    return tuple(np.float32(v) for v in (tot, disp, ent, rep, inter))


def _build_nc(mode: str):
    import concourse.bacc as bacc
    import concourse.tile as tile
    from concourse import mybir

    f32 = mybir.dt.float32
    fin = {"f8e3": mybir.dt.float8e3, "f8e4": mybir.dt.float8e4,
           "f16": mybir.dt.float16}[mode]

    nc = bacc.Bacc("TRN2", target_bir_lowering=False, debug=False,
                   enable_asserts=False, enable_partition_id=False)
    g_dram = nc.dram_tensor("g", (NB * 128, PG * FK), fin, kind="ExternalInput").ap()
    y_dram = nc.dram_tensor("y", (128, NT * YW), fin, kind="ExternalInput").ap()
    out_dram = nc.dram_tensor("out", (128, 2 * YW), f32, kind="ExternalOutput").ap()

    with tile.TileContext(nc) as tc:
        with (
            tc.tile_pool(name="singles", bufs=1) as singles,
            tc.tile_pool(name="gpool", bufs=NB) as gpool,
            tc.tile_pool(name="res", bufs=1) as res,
            tc.tile_pool(name="psacc", bufs=1, space="PSUM") as psacc,
            tc.tile_pool(name="pswarm", bufs=1, space="PSUM") as pswarm,
        ):
            # PE warm-up: dependency-free matmuls keep the array busy while
            # the first DMAs land, so HAM reaches K=8/8 before real work.
            # The memset goes on DVE, whose sequencer is ready earliest.
            wsb = singles.tile([128, 64], f32)
            nc.vector.memset(wsb, 0.0)
            wps = pswarm.tile([64, 64], f32)
            for _ in range(WARM_MM):
                nc.tensor.matmul(wps, wsb, wsb, start=True, stop=True)

            # [Y | 1 | ysq_hi | ysq_lo] resident, host-packed; pure DMA.
            yres = singles.tile([128, NT * YW], fin, name="yres")
            gts = [gpool.tile([128, PG * FK], fin, name=f"g{b}", tag="g")
                   for b in range(NB)]

            def emit_y(lo, hi):
                nc.scalar.dma_start(
                    out=yres[:, lo * YW:hi * YW], in_=y_dram[:, lo * YW:hi * YW]
                )

            def emit_g(b, lo, hi):
                nc.sync.dma_start(
                    out=gts[b][:, lo * FK:hi * FK],
                    in_=g_dram[b * 128:(b + 1) * 128, lo * FK:hi * FK],
                )

            # Two rings: g stream on sync, y stream on scalar (per-ring FIFO
            # keeps g delivery in consumption order).  The first y/g pieces
            # are small so the first matmul's data — and its DMA completion
            # receipt — arrive as early as possible.
            emit_y(0, 2)
            emit_g(0, 0, 2)
            emit_y(2, YCH)
            emit_g(0, 2, PG)
            for b in range(1, NB):
                emit_g(b, 0, PG)
            for j in range(1, NT // YCH):
                emit_y(YCH * j, YCH * (j + 1))

            # phase 1: ps_h[fk, 132] += G_half_s^T @ yext_s over 128 subtiles
            ps0 = psacc.tile([128, YW], f32)
            ps1 = psacc.tile([128, YW], f32)
            for b in range(NB):
                for t in range(PG):
                    s = b * PG + t
                    rhs = yres[:, s * YW:(s + 1) * YW]
                    st, sp = (s == 0), (s == NT - 1)
                    nc.tensor.matmul(
                        ps0, gts[b][:, t * FK:t * FK + 128], rhs, start=st, stop=sp)
                    nc.tensor.matmul(
                        ps1, gts[b][:, t * FK + 128:(t + 1) * FK], rhs, start=st, stop=sp)

            # drain: psum -> sbuf on two engines, out halves on both rings
            sbout = res.tile([128, 2 * YW], f32)
            nc.scalar.copy(sbout[:, 0:YW], ps0)
            nc.vector.tensor_copy(sbout[:, YW:2 * YW], ps1)
            nc.sync.dma_start(out=out_dram[:, 0:YW], in_=sbout[:, 0:YW])
            nc.scalar.dma_start(out=out_dram[:, YW:2 * YW], in_=sbout[:, YW:2 * YW])

    nc.compile()
    return nc


def get_nc(mode: str = "f8e3"):
    if mode not in _NC_CACHE:
        _NC_CACHE[mode] = _build_nc(mode)
    return _NC_CACHE[mode]


def kernel(membership: np.ndarray, teacher_preds: np.ndarray, _trace: bool = False,
           _mode: str = "f8e3"):
    from concourse.bass_utils import run_bass_kernel_spmd

    npdt = _NPDT[_mode]
    y32 = np.asarray(teacher_preds, dtype=np.float32)
    ysq = (y32.astype(np.float64) ** 2).sum(axis=1) / YSQ_SCALE   # exact, host
    hi = ysq.astype(np.float32).astype(npdt)
    lo = (ysq - hi.astype(np.float64)).astype(np.float32).astype(npdt)
    yext = np.zeros((N, YW), dtype=npdt)
    yext[:, 0:C] = y32.astype(npdt)
    yext[:, C] = np.float32(1.0)
    yext[:, C + 1] = hi
    yext[:, C + 2] = lo
    y_packed = _pack_y(yext)

    m = np.asarray(membership, dtype=np.float32).reshape(N, F * K).astype(npdt)

    nc = get_nc(_mode)
    in_maps = []
    for i in range(NCORES):
        in_maps.append({
            "g": _pack_g(m[:, i * FK:(i + 1) * FK]),
            "y": y_packed,
        })
    res = run_bass_kernel_spmd(
        nc, in_maps, core_ids=list(range(NCORES)), trace=_trace,
    )
    parts = np.stack(
        [np.asarray(res.results[i]["out"], dtype=np.float64) for i in range(NCORES)]
    )
    out = _finalize(parts)
    if _trace:
        return out, res
    return out


if __name__ == "__main__":
    rng = np.random.default_rng(0)
    mem = rng.random((N, F, K), dtype=np.float32)
    tp = rng.random((N, C), dtype=np.float32)
    print(kernel(mem, tp))
